# revision 38
# baseline (speedup 1.0000x reference)
"""Trainium2 Bass kernel for MeanAggregator GNN message passing.

Computation (see reference):
  h = tanh(BN_trainmode(features @ W.T + b)) ; out = row-mean over sampled
  neighbor set (deduped membership mask) of h rows.  The linear bias b
  cancels exactly inside train-mode BN (shift-invariant), so it is dropped.

Strategy (8 cores, SPMD), rev9 — gather-free, fp8 DoubleRow stats:
  - Shard OUTPUT rows across cores (512 rows/core).  The host pre-gathers
    the feature rows for each (row, slot) entry: every output row gets
    exactly S=17 slots (pad slots carry weight 0), so each core receives a
    dense [256, 8704] fp16 entry matrix plus a [1, 8704] fp16 weight row.
    (No on-device dma_gather, no output ReduceScatter.)
  - BN batch stats need the full table; only channel sums/sumsq are used,
    so the table shard + W ride in float8e4 (global averaging washes the
    quantization out; measured 2e-3 end-to-end) packed in ONE tensor:
    per partition, k-tile 0 = [W rows 0:128 | table rows 0:128] and
    k-tile 1 = [W rows 128:256 | table rows 128:256].  Each 512-column
    chunk is a single DoubleRow matmul (2 fp8 contraction rows/cycle,
    256-deep reduction in one pass); DVE reduce -> sum, ACT Square
    accum -> sumsq, 4 rotating PSUM banks.
  - Stats exchange: CC AllGather of [128,2] partials + local slot-sum.
    The CC doorbell quiesces every DMA issued before it in program
    order, so the big entry-feature loads are issued AFTER the
    collective.  ACT's SQRT and TANH tables are preloaded with dummy
    ops during the CC window (saves two 1.3us table loads after).
  - Entry pipeline: fp16 W @ xg^T per 512-entry chunk; raw PSUM->fp16
    DVE drain during the CC window; once stats arrive, a fused ACT pass
    tanh(mm*scale + shift) with per-partition scale/bias, DVE multiply
    by the partition-broadcast weight row, and 17-slot segmented
    reduces in 64-row blocks, each streaming its output piece to HBM.
  - Output is [128, 512] (channels x rows) per core; host transposes and
    concatenates.
"""

import sys

for _p in ("/opt/trn_rl_repo", "/root/.axon_site/_ro/trn_rl_repo"):
    if _p not in sys.path:
        sys.path.append(_p)

import ml_dtypes
import numpy as np

import concourse.bass as bass
import concourse.bacc as bacc
import concourse.tile as tile
import concourse.mybir as mybir
from concourse.bass_utils import run_bass_kernel_spmd

F32 = mybir.dt.float32
F16 = mybir.dt.float16
F8 = mybir.dt.float8e4
AF = mybir.ActivationFunctionType
OP = mybir.AluOpType
AX = mybir.AxisListType
PM = mybir.MatmulPerfMode

N_CORES = 8
U, F, E, B = 50000, 256, 128, 4096
S = 17                  # slots per output row (n_nbr_samples + self)
UL = 6272               # per-core table rows for stats (49 * 128)
AW = E + UL             # fp8 pack width per k-tile: [W | table]
R = B // N_CORES        # 512 output rows per core
EN = R * S              # 8704 entries per core (= 17 * 512 exactly)
CH = 512                # entry / table chunk width (one PSUM bank)
RB = 64                 # output block rows (RB*S entries per block)
BN_EPS = 1e-5

U_CHUNKS = [(i * CH, CH) for i in range(UL // CH)]
if UL % CH:
    U_CHUNKS.append((UL - UL % CH, UL % CH))
E_CHUNKS = [(i * CH, CH) for i in range(EN // CH)]
XA_PIECES = [(0, E + 1536), (E + 1536, 1536), (E + 3072, 1536),
             (E + 4608, 1664)]

_CACHE = {}
LAST_RESULTS = None
TRACE = False


def _build():
    if "nc" in _CACHE:
        return _CACHE["nc"]

    nc = bacc.Bacc("TRN2", target_bir_lowering=False, debug=False,
                   enable_asserts=False, num_devices=N_CORES)

    # ---- I/O ----
    xA = nc.dram_tensor("xA", [128, 2 * AW], F8, kind="ExternalInput")
    xgT = nc.dram_tensor("xgT", [F, EN], F16, kind="ExternalInput")
    Wt = nc.dram_tensor("Wt", [F, E], F16, kind="ExternalInput")
    gb = nc.dram_tensor("gb", [E, 2], F32, kind="ExternalInput")
    wrow = nc.dram_tensor("wrow", [128, EN], F16, kind="ExternalInput")
    outT = nc.dram_tensor("outT", [E, R], F32, kind="ExternalOutput")

    # ---- internal DRAM (stats AllGather + warmup rendezvous) ----
    ag_in = nc.dram_tensor("ag_in", [E, 2], F32)
    ag_out = nc.dram_tensor("ag_out", [N_CORES * E, 2], F32,
                            addr_space="Shared")
    wu_in = nc.dram_tensor("wu_in", [1, 2], F32)
    wu_out = nc.dram_tensor("wu_out", [N_CORES, 2], F32,
                            addr_space="Shared")

    RG = [list(range(N_CORES))]
    xA3 = xA.ap().rearrange("p (two m) -> p two m", two=2)

    with tile.TileContext(nc) as tc:
        with (
            tc.tile_pool(name="const", bufs=1) as cpool,
            tc.tile_pool(name="rot", bufs=3) as rot,
        ):
            # ---- warmup collective FIRST (before any dma_start, so its
            # quiesce barrier is empty): rendezvouses the 8 cores at
            # kernel start, absorbing the per-core launch skew that
            # otherwise lands inside the real stats collective ----
            nc.gpsimd.collective_compute(
                "AllGather", OP.bypass, replica_groups=RG,
                ins=[wu_in.ap()], outs=[wu_out.ap()])

            # ---- stats-critical load first: fp8 [W | table] piece 0 ----
            xa = cpool.tile([128, 2, AW], F8, tag="xa")
            p0, pn = XA_PIECES[0]
            nc.sync.dma_start(xa[:, :, p0:p0 + pn], xA3[:, :, p0:p0 + pn])

            wt0 = cpool.tile([128, E], F16, tag="wt0")
            wt1 = cpool.tile([128, E], F16, tag="wt1")
            nc.sync.dma_start(wt0[:], Wt[0:128, :])
            nc.sync.dma_start(wt1[:], Wt[128:256, :])
            gbt = cpool.tile([E, 2], F32, tag="gbt")
            nc.sync.dma_start(gbt[:], gb[:])
            epscol = cpool.tile([E, 1], F32, tag="epscol")
            nc.vector.memset(epscol[:], BN_EPS)

            # remaining table pieces
            for p0, pn in XA_PIECES[1:]:
                nc.sync.dma_start(xa[:, :, p0:p0 + pn], xA3[:, :, p0:p0 + pn])

            n_ch = len(U_CHUNKS)
            musum = cpool.tile([E, n_ch], F32, tag="musum")
            ssq = cpool.tile([E, n_ch], F32, tag="ssq")

            # ---- phase A: fp8 DoubleRow table GEMM -> sum / sumsq ----
            with tc.tile_pool(name="psA", bufs=1, space="PSUM") as psA:
                for ci, (u0, un) in enumerate(U_CHUNKS):
                    ps = psA.tile([128, un], F32, tag=f"ps{ci % 4}")
                    nc.tensor.matmul(
                        ps[:], xa[:, :, 0:E], xa[:, :, E + u0:E + u0 + un],
                        start=True, stop=True, perf_mode=PM.DoubleRow)
                    nc.vector.tensor_reduce(musum[:, ci:ci + 1], ps[:],
                                            axis=AX.X, op=OP.add)
                    sqd = rot.tile([128, un], F16, tag="sqd")
                    nc.scalar.activation(sqd[:], ps[:], AF.Square,
                                         accum_out=ssq[:, ci:ci + 1])

            # ---- stats AllGather; doorbell fires at stats-ready since
            # the entry loads are issued after the collective ----
            stats_sb = cpool.tile([E, 2], F32, tag="stats_sb")
            nc.vector.tensor_reduce(stats_sb[:, 0:1], musum[:], axis=AX.X,
                                    op=OP.add)
            nc.vector.tensor_reduce(stats_sb[:, 1:2], ssq[:], axis=AX.X,
                                    op=OP.add)
            nc.scalar.dma_start(ag_in[:], stats_sb[:])
            nc.gpsimd.collective_compute(
                "AllGather", OP.bypass, replica_groups=RG,
                ins=[ag_in.ap()], outs=[ag_out.ap()])

            # preload ACT tables (SQRT, TANH) during the CC window
            dum = cpool.tile([E, 1], F16, tag="dum")
            nc.scalar.activation(dum[:], epscol[:], AF.Sqrt)
            nc.scalar.activation(dum[:], epscol[:], AF.Tanh)

            # entry features + replicated weight rows (fp16): issued
            # after the collective so the doorbell's quiesce does not
            # cover them; they stream during the CC window
            xg0 = cpool.tile([128, EN], F16, tag="xg0")
            xg1 = cpool.tile([128, EN], F16, tag="xg1")
            nc.sync.dma_start(xg0[:, 0:EN // 2], xgT[0:128, 0:EN // 2])
            nc.sync.dma_start(xg1[:, 0:EN // 2], xgT[128:256, 0:EN // 2])
            nc.sync.dma_start(xg0[:, EN // 2:], xgT[0:128, EN // 2:])
            nc.sync.dma_start(xg1[:, EN // 2:], xgT[128:256, EN // 2:])
            wmt = cpool.tile([128, EN], F16, tag="wmt")
            nc.sync.dma_start(wmt[:], wrow[:])

            # ---- phase B GEMM raw-drains to SBUF fp16 (no stats dep),
            # runs inside the CC window ----
            mmr = cpool.tile([128, EN], F16, tag="mmr")
            with tc.tile_pool(name="psB", bufs=1, space="PSUM") as psB:
                for ci, (e0, en) in enumerate(E_CHUNKS):
                    ps = psB.tile([128, en], F32, tag=f"pb{ci % 4}")
                    nc.tensor.matmul(ps[:], wt0[:], xg0[:, e0:e0 + en],
                                     start=True, stop=False)
                    nc.tensor.matmul(ps[:], wt1[:], xg1[:, e0:e0 + en],
                                     start=False, stop=True)
                    nc.vector.tensor_copy(mmr[:, e0:e0 + en], ps[:])

            # ---- CC result -> slot sum -> per-channel scale/shift ----
            recv = cpool.tile([E, 8, 2], F32, tag="recv")
            nc.sync.dma_start(
                recv[:], ag_out.ap().rearrange("(k p) c -> p k c", p=E))
            stats_g = cpool.tile([E, 2], F32, tag="stats_g")
            nc.vector.tensor_reduce(
                stats_g[:], recv[:].rearrange("p k c -> p c k"),
                axis=AX.X, op=OP.add)

            mu = cpool.tile([E, 1], F32, tag="mu")
            nc.vector.tensor_scalar_mul(mu[:], stats_g[:, 0:1], 1.0 / U)
            ex2 = cpool.tile([E, 1], F32, tag="ex2")
            nc.vector.tensor_scalar_mul(ex2[:], stats_g[:, 1:2], 1.0 / U)
            musq = cpool.tile([E, 1], F32, tag="musq")
            nc.vector.tensor_tensor(musq[:], mu[:], mu[:], op=OP.mult)
            var = cpool.tile([E, 1], F32, tag="var")
            nc.vector.tensor_tensor(var[:], ex2[:], musq[:], op=OP.subtract)
            sd = cpool.tile([E, 1], F32, tag="sd")
            nc.scalar.activation(sd[:], var[:], AF.Sqrt, bias=epscol[:, 0:1])
            rinv = cpool.tile([E, 1], F32, tag="rinv")
            nc.vector.reciprocal(rinv[:], sd[:])
            scale_c = cpool.tile([E, 1], F32, tag="scale_c")
            nc.vector.tensor_tensor(scale_c[:], rinv[:], gbt[:, 0:1],
                                    op=OP.mult)
            msc = cpool.tile([E, 1], F32, tag="msc")
            nc.vector.tensor_tensor(msc[:], mu[:], scale_c[:], op=OP.mult)
            shift_c = cpool.tile([E, 1], F32, tag="shift_c")
            nc.vector.tensor_tensor(shift_c[:], gbt[:, 1:2], msc[:],
                                    op=OP.subtract)

            # ---- post-stats: tanh + weight + segmented reduce + out ----
            hw = cpool.tile([128, EN], F16, tag="hw")
            outsb = cpool.tile([E, R], F32, tag="outsb")
            nblk = R // RB
            ck_after = {(RB * (rb + 1) * S + CH - 1) // CH - 1: rb
                        for rb in range(nblk - 1)}

            def emit_block(rb):
                lo, hi = RB * rb, RB * (rb + 1)
                nc.vector.tensor_reduce(
                    outsb[:, lo:hi],
                    hw[:, lo * S:hi * S].rearrange("p (r s) -> p r s", s=S),
                    axis=AX.X, op=OP.add)
                nc.sync.dma_start(outT[:, lo:hi], outsb[:, lo:hi])

            for ci, (e0, en) in enumerate(E_CHUNKS):
                hn = rot.tile([128, en], F16, tag="hn")
                nc.scalar.activation(hn[:], mmr[:, e0:e0 + en], AF.Tanh,
                                     bias=shift_c[:, 0:1],
                                     scale=scale_c[:, 0:1])
                nc.vector.tensor_tensor(hw[:, e0:e0 + en], hn[:],
                                        wmt[:, e0:e0 + en], op=OP.mult)
                rb = ck_after.get(ci)
                if rb is not None:
                    emit_block(rb)
            emit_block(nblk - 1)

    nc.compile()
    _CACHE["nc"] = nc
    return nc


def _prep_inputs(features, W, gamma, beta, row_idx, col_idx):
    """Host-side sharding: dedup mask entries, lay out 17 slots per output
    row (zero-weight padding), pre-gather entry feature rows per core."""
    features = np.asarray(features, dtype=np.float32)
    W = np.asarray(W, dtype=np.float32)
    gamma = np.asarray(gamma, dtype=np.float32)
    beta = np.asarray(beta, dtype=np.float32)
    row = np.asarray(row_idx).astype(np.int64)
    col = np.asarray(col_idx).astype(np.int64)

    # dedup (row, col) pairs: mask "set" semantics
    key = row * np.int64(U) + col
    order = np.argsort(key, kind="stable")
    sk = key[order]
    keep_s = np.ones(len(sk), dtype=bool)
    keep_s[1:] = sk[1:] != sk[:-1]
    keep = np.zeros(len(key), dtype=bool)
    keep[order] = keep_s
    urow = row[keep]
    ucol = col[keep]
    cnt = np.bincount(urow, minlength=B)

    # slot layout [B, S]: row r's entries in slots 0..cnt-1, rest weight 0
    o = np.argsort(urow, kind="stable")
    r_s = urow[o]
    c_s = ucol[o]
    cstart = np.concatenate([[0], np.cumsum(cnt)]).astype(np.int64)
    pos = np.arange(len(r_s), dtype=np.int64) - cstart[r_s]
    cols_slot = np.zeros((B, S), dtype=np.int64)
    w_slot = np.zeros((B, S), dtype=np.float32)
    cols_slot[r_s, pos] = c_s
    w_slot[r_s, pos] = 1.0 / np.maximum(cnt, 1)[r_s]

    Wt_full = np.ascontiguousarray(W.T).astype(np.float16)
    WT8 = np.ascontiguousarray(W.T).astype(ml_dtypes.float8_e4m3)
    gb_full = np.ascontiguousarray(np.stack([gamma, beta], axis=1))

    in_maps = []
    for k in range(N_CORES):
        cf = cols_slot[k * R:(k + 1) * R].reshape(-1)
        wf = w_slot[k * R:(k + 1) * R].reshape(-1).astype(np.float16)
        xgT_k = np.ascontiguousarray(features[cf].T).astype(np.float16)
        lo, hi = k * UL, min((k + 1) * UL, U)
        xpart = np.zeros((UL, F), dtype=np.float32)
        xpart[:hi - lo] = features[lo:hi]
        xT8 = xpart.T.astype(ml_dtypes.float8_e4m3)   # [256, UL]
        xa = np.zeros((128, 2, AW), dtype=ml_dtypes.float8_e4m3)
        xa[:, 0, :E] = WT8[0:128]
        xa[:, 1, :E] = WT8[128:256]
        xa[:, 0, E:] = xT8[0:128]
        xa[:, 1, E:] = xT8[128:256]
        in_maps.append({
            "xA": np.ascontiguousarray(xa.reshape(128, 2 * AW)),
            "xgT": xgT_k,
            "Wt": Wt_full,
            "gb": gb_full,
            "wrow": np.ascontiguousarray(np.broadcast_to(wf, (128, EN))),
        })
    return in_maps


def kernel(features, W, b, gamma, beta, row_idx, col_idx, B=4096):
    global LAST_RESULTS
    in_maps = _prep_inputs(features, W, gamma, beta, row_idx, col_idx)
    nc = _build()
    res = run_bass_kernel_spmd(nc, in_maps, list(range(N_CORES)), trace=TRACE)
    LAST_RESULTS = res
    out = np.concatenate(
        [np.asarray(res.results[c]["outT"]).T for c in range(N_CORES)],
        axis=0).astype(np.float32)
    return out


# revision 43
# speedup vs baseline: 1.0325x; 1.0325x over previous
"""Trainium2 Bass kernel for MeanAggregator GNN message passing.

Computation (see reference):
  h = tanh(BN_trainmode(features @ W.T + b)) ; out = row-mean over sampled
  neighbor set (deduped membership mask) of h rows.  The linear bias b
  cancels exactly inside train-mode BN (shift-invariant), so it is dropped.

Strategy (8 cores, SPMD), rev9 — gather-free, fp8 DoubleRow stats:
  - Shard OUTPUT rows across cores (512 rows/core).  The host pre-gathers
    the feature rows for each (row, slot) entry: every output row gets
    exactly S=17 slots (pad slots carry weight 0), so each core receives a
    dense [256, 8704] fp16 entry matrix plus a [1, 8704] fp16 weight row.
    (No on-device dma_gather, no output ReduceScatter.)
  - BN batch stats need the full table; only channel sums/sumsq are used,
    so the table shard + W ride in float8e4 (global averaging washes the
    quantization out; measured 2e-3 end-to-end) packed in ONE tensor:
    per partition, k-tile 0 = [W rows 0:128 | table rows 0:128] and
    k-tile 1 = [W rows 128:256 | table rows 128:256].  Each 512-column
    chunk is a single DoubleRow matmul (2 fp8 contraction rows/cycle,
    256-deep reduction in one pass); DVE reduce -> sum, ACT Square
    accum -> sumsq, 4 rotating PSUM banks.
  - Stats exchange: CC AllGather of [128,2] partials + local slot-sum.
    The CC doorbell quiesces every DMA issued before it in program
    order, so the big entry-feature loads are issued AFTER the
    collective.  ACT's SQRT and TANH tables are preloaded with dummy
    ops during the CC window (saves two 1.3us table loads after).
  - Entry pipeline: fp16 W @ xg^T per 512-entry chunk; raw PSUM->fp16
    DVE drain during the CC window; once stats arrive, a fused ACT pass
    tanh(mm*scale + shift) with per-partition scale/bias, DVE multiply
    by the partition-broadcast weight row, and 17-slot segmented
    reduces in 64-row blocks, each streaming its output piece to HBM.
  - Output is [128, 512] (channels x rows) per core; host transposes and
    concatenates.
"""

import sys

for _p in ("/opt/trn_rl_repo", "/root/.axon_site/_ro/trn_rl_repo"):
    if _p not in sys.path:
        sys.path.append(_p)

import ml_dtypes
import numpy as np

import concourse.bass as bass
import concourse.bacc as bacc
import concourse.tile as tile
import concourse.mybir as mybir
from concourse.bass_utils import run_bass_kernel_spmd

F32 = mybir.dt.float32
F16 = mybir.dt.float16
F8 = mybir.dt.float8e4
AF = mybir.ActivationFunctionType
OP = mybir.AluOpType
AX = mybir.AxisListType
PM = mybir.MatmulPerfMode

N_CORES = 8
U, F, E, B = 50000, 256, 128, 4096
S = 17                  # slots per output row (n_nbr_samples + self)
UL = 6272               # per-core table rows for stats (49 * 128)
AW = E + UL             # fp8 pack width per k-tile: [W | table]
R = B // N_CORES        # 512 output rows per core
EN = R * S              # 8704 entries per core (= 17 * 512 exactly)
CH = 512                # entry / table chunk width (one PSUM bank)
RB = 64                 # output block rows (RB*S entries per block)
BN_EPS = 1e-5

U_CHUNKS = [(i * CH, CH) for i in range(UL // CH)]
if UL % CH:
    U_CHUNKS.append((UL - UL % CH, UL % CH))
E_CHUNKS = [(i * CH, CH) for i in range(EN // CH)]
XA_PIECES = [(0, E + 1536), (E + 1536, 1536), (E + 3072, 1536),
             (E + 4608, 1664)]

_CACHE = {}
LAST_RESULTS = None
TRACE = False


def _build():
    if "nc" in _CACHE:
        return _CACHE["nc"]

    nc = bacc.Bacc("TRN2", target_bir_lowering=False, debug=False,
                   enable_asserts=False, num_devices=N_CORES)

    # ---- I/O ----
    xA = nc.dram_tensor("xA", [128, 2 * AW], F8, kind="ExternalInput")
    xgT = nc.dram_tensor("xgT", [F, EN], F16, kind="ExternalInput")
    Wt = nc.dram_tensor("Wt", [F, E], F16, kind="ExternalInput")
    gb = nc.dram_tensor("gb", [E, 2], F32, kind="ExternalInput")
    wrow = nc.dram_tensor("wrow", [128, EN], F16, kind="ExternalInput")
    outT = nc.dram_tensor("outT", [E, R], F16, kind="ExternalOutput")

    # ---- internal DRAM (stats AllGather + warmup rendezvous) ----
    ag_in = nc.dram_tensor("ag_in", [E, 2], F32)
    ag_out = nc.dram_tensor("ag_out", [N_CORES * E, 2], F32,
                            addr_space="Shared")


    RG = [list(range(N_CORES))]
    xA3 = xA.ap().rearrange("p (two m) -> p two m", two=2)

    with tile.TileContext(nc) as tc:
        with (
            tc.tile_pool(name="const", bufs=1) as cpool,
            tc.tile_pool(name="rot", bufs=3) as rot,
        ):
            # (A warmup collective at t=0 was tried: the CC cores have a
            # fixed ~60-85us wakeup after kernel launch regardless of
            # doorbell time, so a warmup only ADDS its own processing to
            # the real collective's completion.  Doorbell-side latency
            # below that wakeup is free slack.)

            # ---- stats-critical load first: fp8 [W | table] piece 0 ----
            xa = cpool.tile([128, 2, AW], F8, tag="xa")
            p0, pn = XA_PIECES[0]
            nc.sync.dma_start(xa[:, :, p0:p0 + pn], xA3[:, :, p0:p0 + pn])

            wt0 = cpool.tile([128, E], F16, tag="wt0")
            wt1 = cpool.tile([128, E], F16, tag="wt1")
            nc.sync.dma_start(wt0[:], Wt[0:128, :])
            nc.sync.dma_start(wt1[:], Wt[128:256, :])
            gbt = cpool.tile([E, 2], F32, tag="gbt")
            nc.sync.dma_start(gbt[:], gb[:])
            epscol = cpool.tile([E, 1], F32, tag="epscol")
            nc.vector.memset(epscol[:], BN_EPS)

            # remaining table pieces
            for p0, pn in XA_PIECES[1:]:
                nc.sync.dma_start(xa[:, :, p0:p0 + pn], xA3[:, :, p0:p0 + pn])

            n_ch = len(U_CHUNKS)
            musum = cpool.tile([E, n_ch], F32, tag="musum")
            ssq = cpool.tile([E, n_ch], F32, tag="ssq")

            # ---- phase A: fp8 DoubleRow table GEMM -> sum / sumsq ----
            with tc.tile_pool(name="psA", bufs=1, space="PSUM") as psA:
                for ci, (u0, un) in enumerate(U_CHUNKS):
                    ps = psA.tile([128, un], F32, tag=f"ps{ci % 4}")
                    nc.tensor.matmul(
                        ps[:], xa[:, :, 0:E], xa[:, :, E + u0:E + u0 + un],
                        start=True, stop=True, perf_mode=PM.DoubleRow)
                    nc.vector.tensor_reduce(musum[:, ci:ci + 1], ps[:],
                                            axis=AX.X, op=OP.add)
                    sqd = rot.tile([128, un], F16, tag="sqd")
                    nc.scalar.activation(sqd[:], ps[:], AF.Square,
                                         accum_out=ssq[:, ci:ci + 1])

            # ---- stats AllGather; doorbell fires at stats-ready since
            # the entry loads are issued after the collective ----
            stats_sb = cpool.tile([E, 2], F32, tag="stats_sb")
            nc.vector.tensor_reduce(stats_sb[:, 0:1], musum[:], axis=AX.X,
                                    op=OP.add)
            nc.vector.tensor_reduce(stats_sb[:, 1:2], ssq[:], axis=AX.X,
                                    op=OP.add)
            nc.scalar.dma_start(ag_in[:], stats_sb[:])
            nc.gpsimd.collective_compute(
                "AllGather", OP.bypass, replica_groups=RG,
                ins=[ag_in.ap()], outs=[ag_out.ap()])

            # preload ACT tables (SQRT, TANH) during the CC window
            dum = cpool.tile([E, 1], F16, tag="dum")
            nc.scalar.activation(dum[:], epscol[:], AF.Sqrt)
            nc.scalar.activation(dum[:], epscol[:], AF.Tanh)

            # entry features + replicated weight rows (fp16): issued
            # after the collective so the doorbell's quiesce does not
            # cover them; they stream during the CC window
            xg0 = cpool.tile([128, EN], F16, tag="xg0")
            xg1 = cpool.tile([128, EN], F16, tag="xg1")
            nc.sync.dma_start(xg0[:, 0:EN // 2], xgT[0:128, 0:EN // 2])
            nc.sync.dma_start(xg1[:, 0:EN // 2], xgT[128:256, 0:EN // 2])
            nc.sync.dma_start(xg0[:, EN // 2:], xgT[0:128, EN // 2:])
            nc.sync.dma_start(xg1[:, EN // 2:], xgT[128:256, EN // 2:])
            wmt = cpool.tile([128, EN], F16, tag="wmt")
            nc.sync.dma_start(wmt[:], wrow[:])

            # ---- phase B GEMM raw-drains to SBUF fp16 (no stats dep),
            # runs inside the CC window ----
            mmr = cpool.tile([128, EN], F16, tag="mmr")
            with tc.tile_pool(name="psB", bufs=1, space="PSUM") as psB:
                for ci, (e0, en) in enumerate(E_CHUNKS):
                    ps = psB.tile([128, en], F32, tag=f"pb{ci % 4}")
                    nc.tensor.matmul(ps[:], wt0[:], xg0[:, e0:e0 + en],
                                     start=True, stop=False)
                    nc.tensor.matmul(ps[:], wt1[:], xg1[:, e0:e0 + en],
                                     start=False, stop=True)
                    nc.vector.tensor_copy(mmr[:, e0:e0 + en], ps[:])

            # ---- CC result -> slot sum -> per-channel scale/shift ----
            recv = cpool.tile([E, 8, 2], F32, tag="recv")
            nc.sync.dma_start(
                recv[:], ag_out.ap().rearrange("(k p) c -> p k c", p=E))
            stats_g = cpool.tile([E, 2], F32, tag="stats_g")
            nc.vector.tensor_reduce(
                stats_g[:], recv[:].rearrange("p k c -> p c k"),
                axis=AX.X, op=OP.add)

            mu = cpool.tile([E, 1], F32, tag="mu")
            nc.vector.tensor_scalar_mul(mu[:], stats_g[:, 0:1], 1.0 / U)
            ex2 = cpool.tile([E, 1], F32, tag="ex2")
            nc.vector.tensor_scalar_mul(ex2[:], stats_g[:, 1:2], 1.0 / U)
            musq = cpool.tile([E, 1], F32, tag="musq")
            nc.vector.tensor_tensor(musq[:], mu[:], mu[:], op=OP.mult)
            var = cpool.tile([E, 1], F32, tag="var")
            nc.vector.tensor_tensor(var[:], ex2[:], musq[:], op=OP.subtract)
            sd = cpool.tile([E, 1], F32, tag="sd")
            nc.scalar.activation(sd[:], var[:], AF.Sqrt, bias=epscol[:, 0:1])
            rinv = cpool.tile([E, 1], F32, tag="rinv")
            nc.vector.reciprocal(rinv[:], sd[:])
            scale_c = cpool.tile([E, 1], F32, tag="scale_c")
            nc.vector.tensor_tensor(scale_c[:], rinv[:], gbt[:, 0:1],
                                    op=OP.mult)
            msc = cpool.tile([E, 1], F32, tag="msc")
            nc.vector.tensor_tensor(msc[:], mu[:], scale_c[:], op=OP.mult)
            shift_c = cpool.tile([E, 1], F32, tag="shift_c")
            nc.vector.tensor_tensor(shift_c[:], gbt[:, 1:2], msc[:],
                                    op=OP.subtract)

            # ---- post-stats: tanh + weight + segmented reduce + out ----
            hw = cpool.tile([128, EN], F16, tag="hw")
            outsb = cpool.tile([E, R], F16, tag="outsb")
            nblk = R // RB
            T_CHUNKS = [(i * 1024, min(1024, EN - i * 1024))
                        for i in range((EN + 1023) // 1024)]
            chunk_end = {}
            for ci, (e0, en) in enumerate(T_CHUNKS):
                chunk_end[ci] = e0 + en
            ck_after = {}
            for rb in range(nblk - 1):
                need = RB * (rb + 1) * S
                ck_after[min(ci for ci in chunk_end
                             if chunk_end[ci] >= need)] = rb

            def emit_block(rb):
                lo, hi = RB * rb, RB * (rb + 1)
                with nc.allow_low_precision("17-term fp16 row sums"):
                    nc.vector.tensor_reduce(
                        outsb[:, lo:hi],
                        hw[:, lo * S:hi * S].rearrange(
                            "p (r s) -> p r s", s=S),
                        axis=AX.X, op=OP.add)
                nc.sync.dma_start(outT[:, lo:hi], outsb[:, lo:hi])

            for ci, (e0, en) in enumerate(T_CHUNKS):
                hn = rot.tile([128, en], F16, tag="hn")
                nc.scalar.activation(hn[:], mmr[:, e0:e0 + en], AF.Tanh,
                                     bias=shift_c[:, 0:1],
                                     scale=scale_c[:, 0:1])
                nc.vector.tensor_tensor(hw[:, e0:e0 + en], hn[:],
                                        wmt[:, e0:e0 + en], op=OP.mult)
                rb = ck_after.get(ci)
                if rb is not None:
                    emit_block(rb)
            emit_block(nblk - 1)

    nc.compile()
    _CACHE["nc"] = nc
    return nc


def _prep_inputs(features, W, gamma, beta, row_idx, col_idx):
    """Host-side sharding: dedup mask entries, lay out 17 slots per output
    row (zero-weight padding), pre-gather entry feature rows per core."""
    features = np.asarray(features, dtype=np.float32)
    W = np.asarray(W, dtype=np.float32)
    gamma = np.asarray(gamma, dtype=np.float32)
    beta = np.asarray(beta, dtype=np.float32)
    row = np.asarray(row_idx).astype(np.int64)
    col = np.asarray(col_idx).astype(np.int64)

    # dedup (row, col) pairs: mask "set" semantics
    key = row * np.int64(U) + col
    order = np.argsort(key, kind="stable")
    sk = key[order]
    keep_s = np.ones(len(sk), dtype=bool)
    keep_s[1:] = sk[1:] != sk[:-1]
    keep = np.zeros(len(key), dtype=bool)
    keep[order] = keep_s
    urow = row[keep]
    ucol = col[keep]
    cnt = np.bincount(urow, minlength=B)

    # slot layout [B, S]: row r's entries in slots 0..cnt-1, rest weight 0
    o = np.argsort(urow, kind="stable")
    r_s = urow[o]
    c_s = ucol[o]
    cstart = np.concatenate([[0], np.cumsum(cnt)]).astype(np.int64)
    pos = np.arange(len(r_s), dtype=np.int64) - cstart[r_s]
    cols_slot = np.zeros((B, S), dtype=np.int64)
    w_slot = np.zeros((B, S), dtype=np.float32)
    cols_slot[r_s, pos] = c_s
    w_slot[r_s, pos] = 1.0 / np.maximum(cnt, 1)[r_s]

    Wt_full = np.ascontiguousarray(W.T).astype(np.float16)
    WT8 = np.ascontiguousarray(W.T).astype(ml_dtypes.float8_e4m3)
    gb_full = np.ascontiguousarray(np.stack([gamma, beta], axis=1))

    in_maps = []
    for k in range(N_CORES):
        cf = cols_slot[k * R:(k + 1) * R].reshape(-1)
        wf = w_slot[k * R:(k + 1) * R].reshape(-1).astype(np.float16)
        xgT_k = np.ascontiguousarray(features[cf].T).astype(np.float16)
        lo, hi = k * UL, min((k + 1) * UL, U)
        xpart = np.zeros((UL, F), dtype=np.float32)
        xpart[:hi - lo] = features[lo:hi]
        xT8 = xpart.T.astype(ml_dtypes.float8_e4m3)   # [256, UL]
        xa = np.zeros((128, 2, AW), dtype=ml_dtypes.float8_e4m3)
        xa[:, 0, :E] = WT8[0:128]
        xa[:, 1, :E] = WT8[128:256]
        xa[:, 0, E:] = xT8[0:128]
        xa[:, 1, E:] = xT8[128:256]
        in_maps.append({
            "xA": np.ascontiguousarray(xa.reshape(128, 2 * AW)),
            "xgT": xgT_k,
            "Wt": Wt_full,
            "gb": gb_full,
            "wrow": np.ascontiguousarray(np.broadcast_to(wf, (128, EN))),
        })
    return in_maps


def kernel(features, W, b, gamma, beta, row_idx, col_idx, B=4096):
    global LAST_RESULTS
    in_maps = _prep_inputs(features, W, gamma, beta, row_idx, col_idx)
    nc = _build()
    res = run_bass_kernel_spmd(nc, in_maps, list(range(N_CORES)), trace=TRACE)
    LAST_RESULTS = res
    out = np.concatenate(
        [np.asarray(res.results[c]["outT"]).T for c in range(N_CORES)],
        axis=0).astype(np.float32)
    return out


# revision 45
# speedup vs baseline: 1.1340x; 1.0983x over previous
"""Trainium2 Bass kernel for MeanAggregator GNN message passing.

Computation (see reference):
  h = tanh(BN_trainmode(features @ W.T + b)) ; out = row-mean over sampled
  neighbor set (deduped membership mask) of h rows.  The linear bias b
  cancels exactly inside train-mode BN (shift-invariant), so it is dropped.

Strategy (8 cores, SPMD), rev11 — gather-free, stats-decoupled tail:
  - Shard OUTPUT rows across cores (512 rows/core).  The host pre-gathers
    the feature rows for each (row, slot) entry: every output row gets
    exactly S=17 slots (pad slots carry weight 0); each core receives a
    dense [256, 8704] fp16 entry matrix, a [128, 8704] replicated weight
    matrix, and a [256, 512] weighted-mean feature matrix (xbar).
  - BN batch stats need the full table; only channel sums/sumsq are used,
    so the table shard + W ride in float8e4 packed per k-tile and each
    512-column chunk is a single DoubleRow matmul.  The CC AllGather of
    the [128,2] partials has a hard floor in this environment: the CC
    cores wake ~60-110us after kernel launch regardless of doorbell.
  - To keep that floor off the compute path, the tanh pipeline runs
    DURING the collective using per-core LOCAL shard stats (a0, b0), and
    the global stats enter only through an exact first-order correction:
      out = out0 + da*(mbar - Q1) + db*(1 - Q0)
    with out0 = sum_s w*y0, Q0 = sum_s w*y0^2, Q1 = sum_s w*y0^2*mm,
    mbar = W @ xbar (exact, since sum_s w*mm is linear), da = a - a0,
    db = b - b0.  Local stats are off by <2.5%, so the dropped
    second-order term contributes ~3e-4 end-to-end (measured).
    Post-collective work collapses to the scale/shift chain + four
    fused per-partition DVE ops + one output DMA.
  - Engine budget in the CC window: ACT does the y0 tanh drain straight
    from PSUM; GpSimd (issued before the collective, whose completion
    drain blocks that queue) does yw/y2w; DVE does raw-mm copies, q,
    and the three 17-slot segmented reduces in 64-row blocks.
  - Output is [128, 512] (channels x rows) fp16 per core; host
    transposes and converts.
"""

import sys

for _p in ("/opt/trn_rl_repo", "/root/.axon_site/_ro/trn_rl_repo"):
    if _p not in sys.path:
        sys.path.append(_p)

import ml_dtypes
import numpy as np

import concourse.bass as bass
import concourse.bacc as bacc
import concourse.tile as tile
import concourse.mybir as mybir
from concourse.bass_utils import run_bass_kernel_spmd

F32 = mybir.dt.float32
F16 = mybir.dt.float16
F8 = mybir.dt.float8e4
AF = mybir.ActivationFunctionType
OP = mybir.AluOpType
AX = mybir.AxisListType
PM = mybir.MatmulPerfMode

N_CORES = 8
U, F, E, B = 50000, 256, 128, 4096
S = 17                  # slots per output row (n_nbr_samples + self)
UL = 6272               # per-core table rows for stats (49 * 128)
AW = E + UL             # fp8 pack width per k-tile: [W | table]
R = B // N_CORES        # 512 output rows per core
EN = R * S              # 8704 entries per core (= 17 * 512 exactly)
CH = 512                # entry / table chunk width (one PSUM bank)
RB = 64                 # output block rows (RB*S entries per block)
BN_EPS = 1e-5

U_CHUNKS = [(i * CH, CH) for i in range(UL // CH)]
if UL % CH:
    U_CHUNKS.append((UL - UL % CH, UL % CH))
E_CHUNKS = [(i * CH, CH) for i in range(EN // CH)]
XA_PIECES = [(0, E + 1536), (E + 1536, 1536), (E + 3072, 1536),
             (E + 4608, 1664)]

_CACHE = {}
LAST_RESULTS = None
TRACE = False


def _build():
    if "nc" in _CACHE:
        return _CACHE["nc"]

    nc = bacc.Bacc("TRN2", target_bir_lowering=False, debug=False,
                   enable_asserts=False, num_devices=N_CORES)

    # ---- I/O ----
    xA = nc.dram_tensor("xA", [128, 2 * AW], F8, kind="ExternalInput")
    xgT = nc.dram_tensor("xgT", [F, EN], F16, kind="ExternalInput")
    xbT = nc.dram_tensor("xbT", [F, R], F16, kind="ExternalInput")
    Wt = nc.dram_tensor("Wt", [F, E], F16, kind="ExternalInput")
    gb = nc.dram_tensor("gb", [E, 4], F32, kind="ExternalInput")
    wrow = nc.dram_tensor("wrow", [128, EN], F16, kind="ExternalInput")
    outT = nc.dram_tensor("outT", [E, R], F16, kind="ExternalOutput")

    # ---- internal DRAM (stats AllGather) ----
    ag_in = nc.dram_tensor("ag_in", [E, 2], F32)
    ag_out = nc.dram_tensor("ag_out", [N_CORES * E, 2], F32,
                            addr_space="Shared")

    RG = [list(range(N_CORES))]
    xA3 = xA.ap().rearrange("p (two m) -> p two m", two=2)

    with tile.TileContext(nc) as tc:
        with (
            tc.tile_pool(name="const", bufs=1) as cpool,
            tc.tile_pool(name="rot", bufs=3) as rot,
        ):
            # ---- stats-critical load first: fp8 [W | table] piece 0 ----
            xa = cpool.tile([128, 2, AW], F8, tag="xa")
            p0, pn = XA_PIECES[0]
            nc.sync.dma_start(xa[:, :, p0:p0 + pn], xA3[:, :, p0:p0 + pn])

            wt0 = cpool.tile([128, E], F16, tag="wt0")
            wt1 = cpool.tile([128, E], F16, tag="wt1")
            nc.sync.dma_start(wt0[:], Wt[0:128, :])
            nc.sync.dma_start(wt1[:], Wt[128:256, :])
            xb0 = cpool.tile([128, R], F16, tag="xb0")
            xb1 = cpool.tile([128, R], F16, tag="xb1")
            nc.sync.dma_start(xb0[:], xbT[0:128, :])
            nc.sync.dma_start(xb1[:], xbT[128:256, :])
            gbt = cpool.tile([E, 4], F32, tag="gbt")
            nc.sync.dma_start(gbt[:], gb[:])
            epscol = cpool.tile([E, 1], F32, tag="epscol")
            nc.vector.memset(epscol[:], BN_EPS)

            # remaining table pieces, then the entry tensors (the latter
            # land inside the CC window; the quiesce barrier of the
            # collective covers only DMAs issued before it, so the entry
            # loads are issued after the collective below)
            for p0, pn in XA_PIECES[1:]:
                nc.sync.dma_start(xa[:, :, p0:p0 + pn], xA3[:, :, p0:p0 + pn])

            n_ch = len(U_CHUNKS)
            musum = cpool.tile([E, n_ch], F32, tag="musum")
            ssq = cpool.tile([E, n_ch], F32, tag="ssq")
            mbar = cpool.tile([E, R], F32, tag="mbar")

            # ---- phase A: fp8 DoubleRow table GEMM -> sum / sumsq;
            # then the exact linear-aggregate GEMM mbar = W @ xbar ----
            with tc.tile_pool(name="psA", bufs=1, space="PSUM") as psA:
                for ci, (u0, un) in enumerate(U_CHUNKS):
                    ps = psA.tile([128, un], F32, tag=f"ps{ci % 4}")
                    nc.tensor.matmul(
                        ps[:], xa[:, :, 0:E], xa[:, :, E + u0:E + u0 + un],
                        start=True, stop=True, perf_mode=PM.DoubleRow)
                    nc.vector.tensor_reduce(musum[:, ci:ci + 1], ps[:],
                                            axis=AX.X, op=OP.add)
                    sqd = rot.tile([128, un], F16, tag="sqd")
                    nc.scalar.activation(sqd[:], ps[:], AF.Square,
                                         accum_out=ssq[:, ci:ci + 1])
                psm = psA.tile([128, R], F32, tag="psm")
                nc.tensor.matmul(psm[:], wt0[:], xb0[:],
                                 start=True, stop=False)
                nc.tensor.matmul(psm[:], wt1[:], xb1[:],
                                 start=False, stop=True)
                nc.vector.tensor_copy(mbar[:], psm[:])

            # ---- my stats partial (for the collective + local stats) ----
            stats_sb = cpool.tile([E, 2], F32, tag="stats_sb")
            nc.vector.tensor_reduce(stats_sb[:, 0:1], musum[:], axis=AX.X,
                                    op=OP.add)
            nc.vector.tensor_reduce(stats_sb[:, 1:2], ssq[:], axis=AX.X,
                                    op=OP.add)
            nc.scalar.dma_start(ag_in[:], stats_sb[:])

            # ---- LOCAL shard stats -> a0, b0 (per-partition columns) ----
            invn = gbt[:, 2:3]
            mu0 = cpool.tile([E, 1], F32, tag="mu0")
            nc.vector.tensor_tensor(mu0[:], stats_sb[:, 0:1], invn,
                                    op=OP.mult)
            ex20 = cpool.tile([E, 1], F32, tag="ex20")
            nc.vector.tensor_tensor(ex20[:], stats_sb[:, 1:2], invn,
                                    op=OP.mult)
            var0 = cpool.tile([E, 1], F32, tag="var0")
            musq0 = cpool.tile([E, 1], F32, tag="musq0")
            nc.vector.tensor_tensor(musq0[:], mu0[:], mu0[:], op=OP.mult)
            nc.vector.tensor_tensor(var0[:], ex20[:], musq0[:],
                                    op=OP.subtract)
            sd0 = cpool.tile([E, 1], F32, tag="sd0")
            nc.scalar.activation(sd0[:], var0[:], AF.Sqrt,
                                 bias=epscol[:, 0:1])
            rinv0 = cpool.tile([E, 1], F32, tag="rinv0")
            nc.vector.reciprocal(rinv0[:], sd0[:])
            a0 = cpool.tile([E, 1], F32, tag="a0")
            nc.vector.tensor_tensor(a0[:], rinv0[:], gbt[:, 0:1], op=OP.mult)
            msc0 = cpool.tile([E, 1], F32, tag="msc0")
            nc.vector.tensor_tensor(msc0[:], mu0[:], a0[:], op=OP.mult)
            b0 = cpool.tile([E, 1], F32, tag="b0")
            nc.vector.tensor_tensor(b0[:], gbt[:, 1:2], msc0[:],
                                    op=OP.subtract)

            # entry features + replicated weights: stream during CC window
            xg0 = cpool.tile([128, EN], F16, tag="xg0")
            xg1 = cpool.tile([128, EN], F16, tag="xg1")
            nc.sync.dma_start(xg0[:, 0:EN // 2], xgT[0:128, 0:EN // 2])
            nc.sync.dma_start(xg1[:, 0:EN // 2], xgT[128:256, 0:EN // 2])
            nc.sync.dma_start(xg0[:, EN // 2:], xgT[0:128, EN // 2:])
            nc.sync.dma_start(xg1[:, EN // 2:], xgT[128:256, EN // 2:])
            wmt = cpool.tile([128, EN], F16, tag="wmt")
            nc.sync.dma_start(wmt[:], wrow[:])

            # ---- phase B inside the CC window: per 512-entry chunk
            # GEMM -> { ACT y0 = tanh(a0*mm+b0), DVE raw copy }, then
            # GpSimd yw/y2w, DVE q, and per-64-row-block reduces ----
            mmr = cpool.tile([128, EN], F16, tag="mmr")
            y0t = cpool.tile([128, EN], F16, tag="y0t")
            ywt = cpool.tile([128, EN], F16, tag="ywt")
            y2wt = cpool.tile([128, EN], F16, tag="y2wt")
            qt = cpool.tile([128, EN], F16, tag="qt")
            out0 = cpool.tile([E, R], F16, tag="out0")
            q0m = cpool.tile([E, R], F16, tag="q0m")
            q1m = cpool.tile([E, R], F16, tag="q1m")

            nblk = R // RB
            ck_after = {}
            for rb in range(nblk - 1):
                need = RB * (rb + 1) * S
                ck_after[(need + CH - 1) // CH - 1] = rb

            def emit_block(rb):
                lo, hi = RB * rb, RB * (rb + 1)
                el, eh = lo * S, hi * S
                with nc.allow_low_precision("17-term fp16 row sums"):
                    for dst, src in ((out0, ywt), (q0m, y2wt), (q1m, qt)):
                        nc.vector.tensor_reduce(
                            dst[:, lo:hi],
                            src[:, el:eh].rearrange("p (r s) -> p r s", s=S),
                            axis=AX.X, op=OP.add)

            with tc.tile_pool(name="psB", bufs=1, space="PSUM") as psB:
                for ci, (e0, en) in enumerate(E_CHUNKS):
                    sl = slice(e0, e0 + en)
                    ps = psB.tile([128, en], F32, tag=f"pb{ci % 4}")
                    nc.tensor.matmul(ps[:], wt0[:], xg0[:, sl],
                                     start=True, stop=False)
                    nc.tensor.matmul(ps[:], wt1[:], xg1[:, sl],
                                     start=False, stop=True)
                    nc.scalar.activation(y0t[:, sl], ps[:], AF.Tanh,
                                         bias=b0[:, 0:1], scale=a0[:, 0:1])
                    nc.vector.tensor_copy(mmr[:, sl], ps[:])
                    nc.gpsimd.tensor_tensor(ywt[:, sl], y0t[:, sl],
                                            wmt[:, sl], op=OP.mult)
                    nc.gpsimd.tensor_tensor(y2wt[:, sl], y0t[:, sl],
                                            ywt[:, sl], op=OP.mult)
                    nc.vector.tensor_tensor(qt[:, sl], y2wt[:, sl],
                                            mmr[:, sl], op=OP.mult)
                    rb = ck_after.get(ci)
                    if rb is not None:
                        emit_block(rb)
                emit_block(nblk - 1)

            # ---- the collective (issued after the gpsimd products so
            # its completion drain does not block them) ----
            nc.gpsimd.collective_compute(
                "AllGather", OP.bypass, replica_groups=RG,
                ins=[ag_in.ap()], outs=[ag_out.ap()])

            # ---- CC result -> global stats -> correction scalars ----
            recv = cpool.tile([E, 8, 2], F32, tag="recv")
            nc.sync.dma_start(
                recv[:], ag_out.ap().rearrange("(k p) c -> p k c", p=E))
            stats_g = cpool.tile([E, 2], F32, tag="stats_g")
            nc.vector.tensor_reduce(
                stats_g[:], recv[:].rearrange("p k c -> p c k"),
                axis=AX.X, op=OP.add)

            mu = cpool.tile([E, 1], F32, tag="mu")
            nc.vector.tensor_scalar_mul(mu[:], stats_g[:, 0:1], 1.0 / U)
            ex2 = cpool.tile([E, 1], F32, tag="ex2")
            nc.vector.tensor_scalar_mul(ex2[:], stats_g[:, 1:2], 1.0 / U)
            musq = cpool.tile([E, 1], F32, tag="musq")
            nc.vector.tensor_tensor(musq[:], mu[:], mu[:], op=OP.mult)
            var = cpool.tile([E, 1], F32, tag="var")
            nc.vector.tensor_tensor(var[:], ex2[:], musq[:], op=OP.subtract)
            sd = cpool.tile([E, 1], F32, tag="sd")
            nc.scalar.activation(sd[:], var[:], AF.Sqrt, bias=epscol[:, 0:1])
            rinv = cpool.tile([E, 1], F32, tag="rinv")
            nc.vector.reciprocal(rinv[:], sd[:])
            a_g = cpool.tile([E, 1], F32, tag="a_g")
            nc.vector.tensor_tensor(a_g[:], rinv[:], gbt[:, 0:1], op=OP.mult)
            msc = cpool.tile([E, 1], F32, tag="msc")
            nc.vector.tensor_tensor(msc[:], mu[:], a_g[:], op=OP.mult)
            b_g = cpool.tile([E, 1], F32, tag="b_g")
            nc.vector.tensor_tensor(b_g[:], gbt[:, 1:2], msc[:],
                                    op=OP.subtract)

            da = cpool.tile([E, 1], F32, tag="da")
            nc.vector.tensor_tensor(da[:], a_g[:], a0[:], op=OP.subtract)
            db = cpool.tile([E, 1], F32, tag="db")
            nc.vector.tensor_tensor(db[:], b_g[:], b0[:], op=OP.subtract)
            nda = cpool.tile([E, 1], F32, tag="nda")
            nc.vector.tensor_scalar_mul(nda[:], da[:], -1.0)
            ndb = cpool.tile([E, 1], F32, tag="ndb")
            nc.vector.tensor_scalar_mul(ndb[:], db[:], -1.0)

            # out = out0 + da*(mbar - Q1) + db*(1 - Q0)
            s1 = cpool.tile([E, R], F32, tag="s1")
            nc.vector.scalar_tensor_tensor(s1[:], q1m[:], nda[:, 0:1],
                                           out0[:], op0=OP.mult, op1=OP.add)
            s2 = cpool.tile([E, R], F32, tag="s2")
            nc.vector.scalar_tensor_tensor(s2[:], mbar[:], da[:, 0:1],
                                           s1[:], op0=OP.mult, op1=OP.add)
            s3 = cpool.tile([E, R], F32, tag="s3")
            nc.vector.scalar_tensor_tensor(s3[:], q0m[:], ndb[:, 0:1],
                                           s2[:], op0=OP.mult, op1=OP.add)
            outsb = cpool.tile([E, R], F16, tag="outsb")
            nc.vector.tensor_scalar_add(outsb[:], s3[:], db[:, 0:1])

            nc.sync.dma_start(outT.ap(), outsb[:])

    nc.compile()
    _CACHE["nc"] = nc
    return nc


def _prep_inputs(features, W, gamma, beta, row_idx, col_idx):
    """Host-side sharding: dedup mask entries, lay out 17 slots per output
    row (zero-weight padding), pre-gather entry feature rows per core."""
    features = np.asarray(features, dtype=np.float32)
    W = np.asarray(W, dtype=np.float32)
    gamma = np.asarray(gamma, dtype=np.float32)
    beta = np.asarray(beta, dtype=np.float32)
    row = np.asarray(row_idx).astype(np.int64)
    col = np.asarray(col_idx).astype(np.int64)

    # dedup (row, col) pairs: mask "set" semantics
    key = row * np.int64(U) + col
    order = np.argsort(key, kind="stable")
    sk = key[order]
    keep_s = np.ones(len(sk), dtype=bool)
    keep_s[1:] = sk[1:] != sk[:-1]
    keep = np.zeros(len(key), dtype=bool)
    keep[order] = keep_s
    urow = row[keep]
    ucol = col[keep]
    cnt = np.bincount(urow, minlength=B)

    # slot layout [B, S]: row r's entries in slots 0..cnt-1, rest weight 0
    o = np.argsort(urow, kind="stable")
    r_s = urow[o]
    c_s = ucol[o]
    cstart = np.concatenate([[0], np.cumsum(cnt)]).astype(np.int64)
    pos = np.arange(len(r_s), dtype=np.int64) - cstart[r_s]
    cols_slot = np.zeros((B, S), dtype=np.int64)
    w_slot = np.zeros((B, S), dtype=np.float32)
    cols_slot[r_s, pos] = c_s
    w_slot[r_s, pos] = 1.0 / np.maximum(cnt, 1)[r_s]

    feats16 = features.astype(np.float16)
    Wt_full = np.ascontiguousarray(W.T).astype(np.float16)
    WT8 = np.ascontiguousarray(W.T).astype(ml_dtypes.float8_e4m3)

    in_maps = []
    for k in range(N_CORES):
        rows = slice(k * R, (k + 1) * R)
        cf = cols_slot[rows].reshape(-1)
        wf = w_slot[rows].reshape(-1).astype(np.float16)
        xg = feats16[cf]                                  # [EN, F] fp16
        xgT_k = np.ascontiguousarray(xg.T)
        # exact weighted-mean features per output row (same fp16 inputs)
        xbar = np.einsum("rsf,rs->rf",
                         xg.astype(np.float32).reshape(R, S, F),
                         w_slot[rows])                    # [R, F]
        xbT_k = np.ascontiguousarray(xbar.T).astype(np.float16)
        lo, hi = k * UL, min((k + 1) * UL, U)
        xpart = np.zeros((UL, F), dtype=np.float32)
        xpart[:hi - lo] = features[lo:hi]
        xT8 = xpart.T.astype(ml_dtypes.float8_e4m3)
        xa = np.zeros((128, 2, AW), dtype=ml_dtypes.float8_e4m3)
        xa[:, 0, :E] = WT8[0:128]
        xa[:, 1, :E] = WT8[128:256]
        xa[:, 0, E:] = xT8[0:128]
        xa[:, 1, E:] = xT8[128:256]
        gb4 = np.stack([gamma, beta,
                        np.full(E, 1.0 / (hi - lo), np.float32),
                        np.zeros(E, np.float32)], axis=1)
        in_maps.append({
            "xA": np.ascontiguousarray(xa.reshape(128, 2 * AW)),
            "xgT": xgT_k,
            "xbT": xbT_k,
            "Wt": Wt_full,
            "gb": np.ascontiguousarray(gb4),
            "wrow": np.ascontiguousarray(np.broadcast_to(wf, (128, EN))),
        })
    return in_maps


def kernel(features, W, b, gamma, beta, row_idx, col_idx, B=4096):
    global LAST_RESULTS
    in_maps = _prep_inputs(features, W, gamma, beta, row_idx, col_idx)
    nc = _build()
    res = run_bass_kernel_spmd(nc, in_maps, list(range(N_CORES)), trace=TRACE)
    LAST_RESULTS = res
    out = np.concatenate(
        [np.asarray(res.results[c]["outT"]).astype(np.float32).T
         for c in range(N_CORES)],
        axis=0)
    return out
